# revision 4
# baseline (speedup 1.0000x reference)
"""MoE MLP (GPT-2 style experts, top-2 routing) on 8 Trainium2 NeuronCores.

Strategy (expert-parallel, per sharding hint):
  - Host: router matmul + softmax + top-2 + renormalize (tiny: N x 1024 @ 1024 x 8).
  - Host: dispatch tokens by expert id -> per-core gathered token block (all-to-all
    realized at the shard step), transposed to [C, M] so the device kernel only
    does natural-layout matmuls.
  - Device (core e): outT = w_proj[e].T @ gelu(w_fc[e].T @ xT + b_fc[e])
    computed as tiled PE matmuls, bf16 inputs with f32 PSUM accumulation.
    Both weight matrices stay resident in SBUF (bf16).
  - Host: combine: out[tok] += gate * (y + b_proj[e]) for each routed pair.
"""

import functools
import os

import numpy as np

import concourse.bacc as bacc
import concourse.mybir as mybir
import concourse.tile as tile
from concourse.bass_utils import run_bass_kernel_spmd

N_EMBD = 1024
D_FF = 4096
N_EXPERTS = 8
TOP_K = 2
N_CORES = 8
P = 128
KT = N_EMBD // P      # 8 k-tiles (contraction over n_embd)
FT = D_FF // P        # 32 ff-tiles (contraction over d_ff for proj)
CT = N_EMBD // P      # 8 output-channel tiles
MSZ = 512             # moving (token) tile width

DT16 = mybir.dt.float16
F32 = mybir.dt.float32


def _m_tiles(M):
    out = []
    m0 = 0
    while m0 < M:
        out.append((m0, min(MSZ, M - m0)))
        m0 += MSZ
    return out


@functools.lru_cache(maxsize=4)
def _build(M, repeat=1, act_identity=False):
    """Bass program: per-core dense expert MLP over M gathered tokens."""
    act_fn = (mybir.ActivationFunctionType.Identity if act_identity
              else mybir.ActivationFunctionType.Gelu)
    nc = bacc.Bacc("TRN2", target_bir_lowering=False, debug=False)

    xT = nc.dram_tensor("xT", [KT, P, M], DT16, kind="ExternalInput")
    wfc = nc.dram_tensor("w_fc", [KT, P, D_FF], DT16, kind="ExternalInput")
    bfcT = nc.dram_tensor("b_fcT", [P, FT], F32, kind="ExternalInput")
    wproj = nc.dram_tensor("w_proj", [FT, P, N_EMBD], DT16, kind="ExternalInput")
    outT = nc.dram_tensor("outT", [CT, P, M], F32, kind="ExternalOutput")

    with tile.TileContext(nc) as tc:
        with tc.tile_pool(name="weights", bufs=1) as wpool, \
             tc.tile_pool(name="xp", bufs=2) as xpool, \
             tc.tile_pool(name="hp", bufs=1) as hpool, \
             tc.tile_pool(name="op", bufs=4) as opool, \
             tc.tile_pool(name="psA", bufs=3, space="PSUM") as psA, \
             tc.tile_pool(name="psB", bufs=3, space="PSUM") as psB:

            wfc_sb = wpool.tile([P, KT, D_FF], DT16)
            for k in range(KT):
                nc.sync.dma_start(wfc_sb[:, k, :], wfc[k, :, :])
            bfc_sb = wpool.tile([P, FT], F32)
            nc.sync.dma_start(bfc_sb[:, :], bfcT[:, :])
            wproj_sb = wpool.tile([P, FT, N_EMBD], DT16)
            for f in range(FT):
                nc.sync.dma_start(wproj_sb[:, f, :], wproj[f, :, :])

            for _r in range(repeat):
                for m0, msz in _m_tiles(M):
                    x_sb = xpool.tile([P, KT, MSZ], DT16, tag="x")
                    for k in range(KT):
                        nc.sync.dma_start(x_sb[:, k, :msz], xT[k, :, m0:m0 + msz])

                    hT_sb = hpool.tile([P, FT, MSZ], DT16, tag="h")
                    for f in range(FT):
                        ps = psA.tile([P, MSZ], F32, tag="psA")
                        for k in range(KT):
                            nc.tensor.matmul(
                                ps[:, :msz],
                                wfc_sb[:, k, f * P:(f + 1) * P],
                                x_sb[:, k, :msz],
                                start=(k == 0),
                                stop=(k == KT - 1),
                            )
                        nc.scalar.activation(
                            hT_sb[:, f, :msz], ps[:, :msz],
                            act_fn,
                            bias=bfc_sb[:, f:f + 1],
                        )

                    for c in range(CT):
                        ps2 = psB.tile([P, MSZ], F32, tag="psB")
                        for f in range(FT):
                            nc.tensor.matmul(
                                ps2[:, :msz],
                                wproj_sb[:, f, c * P:(c + 1) * P],
                                hT_sb[:, f, :msz],
                                start=(f == 0),
                                stop=(f == FT - 1),
                            )
                        o_sb = opool.tile([P, MSZ], F32, tag="o")
                        nc.vector.tensor_copy(o_sb[:, :msz], ps2[:, :msz])
                        nc.sync.dma_start(outT[c, :, m0:m0 + msz], o_sb[:, :msz])

    nc.compile()
    return nc


def _route(x_flat, router_w):
    """Top-2 routing, matching the reference numerics (f32)."""
    N = x_flat.shape[0]
    logits = x_flat @ router_w.T                      # [N, E]
    logits -= logits.max(axis=-1, keepdims=True)
    p = np.exp(logits)
    p /= p.sum(axis=-1, keepdims=True)
    rows = np.arange(N)
    i1 = p.argmax(axis=-1)
    p1 = p[rows, i1]
    pm = p.copy()
    pm[rows, i1] = -1.0
    i2 = pm.argmax(axis=-1)
    p2 = p[rows, i2]
    s = p1 + p2 + 1e-9
    return i1, i2, p1 / s, p2 / s


def kernel(x, router_w, w_fc, b_fc, w_proj, b_proj):
    x = np.asarray(x, dtype=np.float32)
    router_w = np.asarray(router_w, dtype=np.float32)
    w_fc = np.asarray(w_fc, dtype=np.float32)
    b_fc = np.asarray(b_fc, dtype=np.float32)
    w_proj = np.asarray(w_proj, dtype=np.float32)
    b_proj = np.asarray(b_proj, dtype=np.float32)

    B, T, C = x.shape
    x_flat = x.reshape(-1, C)
    N = x_flat.shape[0]

    i1, i2, g1, g2 = _route(x_flat, router_w)

    idxs, gates = [], []
    for e in range(N_EXPERTS):
        mask = (i1 == e) | (i2 == e)
        idx = np.flatnonzero(mask)
        g = np.where(i1[idx] == e, g1[idx], g2[idx]).astype(np.float32)
        idxs.append(idx)
        gates.append(g)

    max_cnt = max(len(ix) for ix in idxs)
    M = max(P, ((max_cnt + P - 1) // P) * P)

    repeat = int(os.environ.get("MOE_KERNEL_REPEAT", "1"))
    nc = _build(M, repeat)

    in_maps = []
    for e in range(N_EXPERTS):
        idx = idxs[e]
        xg = np.zeros((M, C), dtype=np.float32)
        xg[: len(idx)] = x_flat[idx]
        xT = np.ascontiguousarray(xg.T).reshape(KT, P, M).astype(np.float16)
        in_maps.append({
            "xT": xT,
            "w_fc": w_fc[e].reshape(KT, P, D_FF).astype(np.float16),
            "b_fcT": np.ascontiguousarray(b_fc[e].reshape(FT, P).T),
            "w_proj": w_proj[e].reshape(FT, P, N_EMBD).astype(np.float16),
        })

    res = run_bass_kernel_spmd(nc, in_maps, core_ids=list(range(N_CORES)))

    out_flat = np.zeros((N, C), dtype=np.float32)
    for e in range(N_EXPERTS):
        idx = idxs[e]
        yT = res.results[e]["outT"].reshape(C, M)       # [C, M]
        y = yT.T[: len(idx)]                            # [n_e, C]
        out_flat[idx] += gates[e][:, None] * (y + b_proj[e])

    return out_flat.reshape(B, T, C)


# revision 8
# speedup vs baseline: 535.4154x; 535.4154x over previous
"""MoE MLP (GPT-2 style experts, top-2 routing) on 8 Trainium2 NeuronCores.

Strategy (expert-parallel, per sharding hint):
  - Host: router matmul + softmax + top-2 + renormalize (tiny: N x 1024 @ 1024 x 8).
  - Host: dispatch tokens by expert id -> per-core gathered token block (all-to-all
    realized at the shard step), transposed to [C, M] so the device kernel only
    does natural-layout matmuls.
  - Device (core e): outT = w_proj[e].T @ gelu(w_fc[e].T @ xT + b_fc[e])
    computed as tiled PE matmuls, fp16 inputs with f32 PSUM accumulation.
    Both weight matrices stay resident in SBUF (fp16).
  - Host: combine: out[tok] += gate * (y + b_proj[e]) for each routed pair.
"""

import functools
import os

import numpy as np

import concourse.bacc as bacc
import concourse.mybir as mybir
import concourse.tile as tile
from concourse.bass_utils import run_bass_kernel_spmd

N_EMBD = 1024
D_FF = 4096
N_EXPERTS = 8
TOP_K = 2
N_CORES = 8
P = 128
KT = N_EMBD // P      # 8 k-tiles (contraction over n_embd)
FT = D_FF // P        # 32 ff-tiles (contraction over d_ff for proj)
CT = N_EMBD // P      # 8 output-channel tiles
MSZ = 512             # moving (token) tile width

DT16 = mybir.dt.float16
F32 = mybir.dt.float32


def _m_tiles(M, msz=MSZ):
    out = []
    m0 = 0
    while m0 < M:
        out.append((m0, min(msz, M - m0)))
        m0 += msz
    return out


@functools.lru_cache(maxsize=8)
def _build(M, repeat=1, act_identity=False, msz=MSZ, psa_bufs=3, psb_bufs=3,
           x_bufs=2, h_bufs=1, o_bufs=4):
    """Bass program: per-core dense expert MLP over M gathered tokens."""
    act_fn = (mybir.ActivationFunctionType.Identity if act_identity
              else mybir.ActivationFunctionType.Gelu)
    nc = bacc.Bacc("TRN2", target_bir_lowering=False, debug=False)

    xT = nc.dram_tensor("xT", [KT, P, M], DT16, kind="ExternalInput")
    wfc = nc.dram_tensor("w_fc", [KT, P, D_FF], DT16, kind="ExternalInput")
    bfcT = nc.dram_tensor("b_fcT", [P, FT], F32, kind="ExternalInput")
    wproj = nc.dram_tensor("w_proj", [FT, P, N_EMBD], DT16, kind="ExternalInput")
    outT = nc.dram_tensor("outT", [CT, P, M], F32, kind="ExternalOutput")

    with tile.TileContext(nc) as tc:
        with tc.tile_pool(name="weights", bufs=1) as wpool, \
             tc.tile_pool(name="xp", bufs=x_bufs) as xpool, \
             tc.tile_pool(name="hp", bufs=h_bufs) as hpool, \
             tc.tile_pool(name="op", bufs=o_bufs) as opool, \
             tc.tile_pool(name="psA", bufs=psa_bufs, space="PSUM") as psA, \
             tc.tile_pool(name="psB", bufs=psb_bufs, space="PSUM") as psB:

            def load_x(m0, mw):
                x_sb = xpool.tile([P, KT, msz], DT16, tag="x", name="x_sb")
                for k in range(KT):
                    nc.sync.dma_start(x_sb[:, k, :mw], xT[k, :, m0:m0 + mw])
                return x_sb

            tiles = _m_tiles(M, msz)
            # First token tile queued ahead of the weights so the PE can
            # start as soon as the first w_fc column-chunk lands.
            pre_x = load_x(*tiles[0])

            wfc_sb = wpool.tile([P, KT, D_FF], DT16)
            CHUNK = 1024
            for c0 in range(0, D_FF, CHUNK):
                for k in range(KT):
                    nc.sync.dma_start(
                        wfc_sb[:, k, c0:c0 + CHUNK], wfc[k, :, c0:c0 + CHUNK]
                    )
            bfc_sb = wpool.tile([P, FT], F32)
            nc.sync.dma_start(bfc_sb[:, :], bfcT[:, :])
            wproj_sb = wpool.tile([P, FT, N_EMBD], DT16)
            for f in range(FT):
                nc.sync.dma_start(wproj_sb[:, f, :], wproj[f, :, :])

            for _r in range(repeat):
                for ti, (m0, mw) in enumerate(tiles):
                    x_sb = pre_x if (_r == 0 and ti == 0) else load_x(m0, mw)

                    hT_sb = hpool.tile([P, FT, msz], DT16, tag="h")
                    for f in range(FT):
                        ps = psA.tile([P, msz], F32, tag="psA")
                        for k in range(KT):
                            nc.tensor.matmul(
                                ps[:, :mw],
                                wfc_sb[:, k, f * P:(f + 1) * P],
                                x_sb[:, k, :mw],
                                start=(k == 0),
                                stop=(k == KT - 1),
                            )
                        nc.scalar.activation(
                            hT_sb[:, f, :mw], ps[:, :mw],
                            act_fn,
                            bias=bfc_sb[:, f:f + 1],
                        )

                    for c in range(CT):
                        ps2 = psB.tile([P, msz], F32, tag="psB")
                        for f in range(FT):
                            nc.tensor.matmul(
                                ps2[:, :mw],
                                wproj_sb[:, f, c * P:(c + 1) * P],
                                hT_sb[:, f, :mw],
                                start=(f == 0),
                                stop=(f == FT - 1),
                            )
                        o_sb = opool.tile([P, msz], F32, tag="o")
                        nc.vector.tensor_copy(o_sb[:, :mw], ps2[:, :mw])
                        nc.sync.dma_start(outT[c, :, m0:m0 + mw], o_sb[:, :mw])

    nc.compile()
    return nc


def _route(x_flat, router_w):
    """Top-2 routing, matching the reference numerics (f32)."""
    N = x_flat.shape[0]
    logits = x_flat @ router_w.T                      # [N, E]
    logits -= logits.max(axis=-1, keepdims=True)
    p = np.exp(logits)
    p /= p.sum(axis=-1, keepdims=True)
    rows = np.arange(N)
    i1 = p.argmax(axis=-1)
    p1 = p[rows, i1]
    pm = p.copy()
    pm[rows, i1] = -1.0
    i2 = pm.argmax(axis=-1)
    p2 = p[rows, i2]
    s = p1 + p2 + 1e-9
    return i1, i2, p1 / s, p2 / s


def kernel(x, router_w, w_fc, b_fc, w_proj, b_proj):
    x = np.asarray(x, dtype=np.float32)
    router_w = np.asarray(router_w, dtype=np.float32)
    w_fc = np.asarray(w_fc, dtype=np.float32)
    b_fc = np.asarray(b_fc, dtype=np.float32)
    w_proj = np.asarray(w_proj, dtype=np.float32)
    b_proj = np.asarray(b_proj, dtype=np.float32)

    B, T, C = x.shape
    x_flat = x.reshape(-1, C)
    N = x_flat.shape[0]

    i1, i2, g1, g2 = _route(x_flat, router_w)

    idxs, gates = [], []
    for e in range(N_EXPERTS):
        mask = (i1 == e) | (i2 == e)
        idx = np.flatnonzero(mask)
        g = np.where(i1[idx] == e, g1[idx], g2[idx]).astype(np.float32)
        idxs.append(idx)
        gates.append(g)

    max_cnt = max(len(ix) for ix in idxs)
    M = max(P, ((max_cnt + P - 1) // P) * P)

    repeat = int(os.environ.get("MOE_KERNEL_REPEAT", "1"))
    nc = _build(M, repeat)

    in_maps = []
    for e in range(N_EXPERTS):
        idx = idxs[e]
        xg = np.zeros((M, C), dtype=np.float32)
        xg[: len(idx)] = x_flat[idx]
        xT = np.ascontiguousarray(xg.T).reshape(KT, P, M).astype(np.float16)
        in_maps.append({
            "xT": xT,
            "w_fc": w_fc[e].reshape(KT, P, D_FF).astype(np.float16),
            "b_fcT": np.ascontiguousarray(b_fc[e].reshape(FT, P).T),
            "w_proj": w_proj[e].reshape(FT, P, N_EMBD).astype(np.float16),
        })

    res = run_bass_kernel_spmd(nc, in_maps, core_ids=list(range(N_CORES)))

    out_flat = np.zeros((N, C), dtype=np.float32)
    for e in range(N_EXPERTS):
        idx = idxs[e]
        yT = res.results[e]["outT"].reshape(C, M)       # [C, M]
        y = yT.T[: len(idx)]                            # [n_e, C]
        out_flat[idx] += gates[e][:, None] * (y + b_proj[e])

    return out_flat.reshape(B, T, C)


# revision 10
# speedup vs baseline: 562.9967x; 1.0515x over previous
"""MoE MLP (GPT-2 style experts, top-2 routing) on 8 Trainium2 NeuronCores.

Strategy (expert-parallel, per sharding hint):
  - Host: router matmul + softmax + top-2 + renormalize (tiny: N x 1024 @ 1024 x 8).
  - Host: dispatch tokens by expert id -> per-core gathered token block (all-to-all
    realized at the shard step), transposed to [C, M] so the device kernel only
    does natural-layout matmuls.
  - Device (core e): outT = w_proj[e].T @ gelu(w_fc[e].T @ xT + b_fc[e])
    computed as tiled PE matmuls, fp16 inputs with f32 PSUM accumulation.
    Both weight matrices stay resident in SBUF (fp16).
  - Host: combine: out[tok] += gate * (y + b_proj[e]) for each routed pair.
"""

import functools
import os

import numpy as np

import concourse.bacc as bacc
import concourse.mybir as mybir
import concourse.tile as tile
from concourse.bass_utils import run_bass_kernel_spmd

N_EMBD = 1024
D_FF = 4096
N_EXPERTS = 8
TOP_K = 2
N_CORES = 8
P = 128
KT = N_EMBD // P      # 8 k-tiles (contraction over n_embd)
FT = D_FF // P        # 32 ff-tiles (contraction over d_ff for proj)
CT = N_EMBD // P      # 8 output-channel tiles
MSZ = 512             # moving (token) tile width

DT16 = mybir.dt.float16
F32 = mybir.dt.float32


def _m_tiles(M, msz=MSZ):
    out = []
    m0 = 0
    while m0 < M:
        out.append((m0, min(msz, M - m0)))
        m0 += msz
    return out


@functools.lru_cache(maxsize=8)
def _build(M, repeat=1, act_identity=False, msz=MSZ, psa_bufs=3, psb_bufs=3,
           x_bufs=2, h_bufs=1, o_bufs=4, weights_in_loop=False):
    """Bass program: per-core dense expert MLP over M gathered tokens."""
    act_fn = (mybir.ActivationFunctionType.Identity if act_identity
              else mybir.ActivationFunctionType.Gelu)
    nc = bacc.Bacc("TRN2", target_bir_lowering=False, debug=False)

    xT = nc.dram_tensor("xT", [KT, P, M], DT16, kind="ExternalInput")
    wfc = nc.dram_tensor("w_fc", [KT, P, D_FF], DT16, kind="ExternalInput")
    bfcT = nc.dram_tensor("b_fcT", [P, FT], F32, kind="ExternalInput")
    wproj = nc.dram_tensor("w_proj", [FT, P, N_EMBD], DT16, kind="ExternalInput")
    outT = nc.dram_tensor("outT", [CT, P, M], F32, kind="ExternalOutput")

    with tile.TileContext(nc) as tc:
        with tc.tile_pool(name="weights", bufs=1) as wpool, \
             tc.tile_pool(name="xp", bufs=x_bufs) as xpool, \
             tc.tile_pool(name="hp", bufs=h_bufs) as hpool, \
             tc.tile_pool(name="op", bufs=o_bufs) as opool, \
             tc.tile_pool(name="psA", bufs=psa_bufs, space="PSUM") as psA, \
             tc.tile_pool(name="psB", bufs=psb_bufs, space="PSUM") as psB:

            def load_x(m0, mw):
                x_sb = xpool.tile([P, KT, msz], DT16, tag="x", name="x_sb")
                for k in range(KT):
                    nc.sync.dma_start(x_sb[:, k, :mw], xT[k, :, m0:m0 + mw])
                return x_sb

            tiles = _m_tiles(M, msz)
            # First token tile queued ahead of the weights so the PE can
            # start as soon as the first w_fc column-chunk lands.
            pre_x = load_x(*tiles[0])

            def load_weights():
                wfc_sb = wpool.tile([P, KT, D_FF], DT16, tag="wfc",
                                    name="wfc_sb")
                CHUNK = 1024
                for c0 in range(0, D_FF, CHUNK):
                    for k in range(KT):
                        nc.sync.dma_start(
                            wfc_sb[:, k, c0:c0 + CHUNK],
                            wfc[k, :, c0:c0 + CHUNK]
                        )
                bfc_sb = wpool.tile([P, FT], F32, tag="bfc", name="bfc_sb")
                nc.sync.dma_start(bfc_sb[:, :], bfcT[:, :])
                wproj_sb = wpool.tile([P, FT, N_EMBD], DT16, tag="wproj",
                                      name="wproj_sb")
                for f in range(FT):
                    nc.sync.dma_start(wproj_sb[:, f, :], wproj[f, :, :])
                return wfc_sb, bfc_sb, wproj_sb

            if not weights_in_loop:
                wfc_sb, bfc_sb, wproj_sb = load_weights()

            for _r in range(repeat):
                if weights_in_loop:
                    wfc_sb, bfc_sb, wproj_sb = load_weights()
                for ti, (m0, mw) in enumerate(tiles):
                    x_sb = pre_x if (_r == 0 and ti == 0) else load_x(m0, mw)

                    hT_sb = hpool.tile([P, FT, msz], DT16, tag="h")
                    for f in range(FT):
                        ps = psA.tile([P, msz], F32, tag="psA")
                        for k in range(KT):
                            nc.tensor.matmul(
                                ps[:, :mw],
                                wfc_sb[:, k, f * P:(f + 1) * P],
                                x_sb[:, k, :mw],
                                start=(k == 0),
                                stop=(k == KT - 1),
                            )
                        nc.scalar.activation(
                            hT_sb[:, f, :mw], ps[:, :mw],
                            act_fn,
                            bias=bfc_sb[:, f:f + 1],
                        )

                    for c in range(CT):
                        ps2 = psB.tile([P, msz], F32, tag="psB")
                        for f in range(FT):
                            nc.tensor.matmul(
                                ps2[:, :mw],
                                wproj_sb[:, f, c * P:(c + 1) * P],
                                hT_sb[:, f, :mw],
                                start=(f == 0),
                                stop=(f == FT - 1),
                            )
                        o_sb = opool.tile([P, msz], F32, tag="o")
                        nc.vector.tensor_copy(o_sb[:, :mw], ps2[:, :mw])
                        nc.sync.dma_start(outT[c, :, m0:m0 + mw], o_sb[:, :mw])

    nc.compile()
    return nc


def _route(x_flat, router_w):
    """Top-2 routing, matching the reference numerics (f32)."""
    N = x_flat.shape[0]
    logits = x_flat @ router_w.T                      # [N, E]
    logits -= logits.max(axis=-1, keepdims=True)
    p = np.exp(logits)
    p /= p.sum(axis=-1, keepdims=True)
    rows = np.arange(N)
    i1 = p.argmax(axis=-1)
    p1 = p[rows, i1]
    pm = p.copy()
    pm[rows, i1] = -1.0
    i2 = pm.argmax(axis=-1)
    p2 = p[rows, i2]
    s = p1 + p2 + 1e-9
    return i1, i2, p1 / s, p2 / s


def kernel(x, router_w, w_fc, b_fc, w_proj, b_proj):
    x = np.asarray(x, dtype=np.float32)
    router_w = np.asarray(router_w, dtype=np.float32)
    w_fc = np.asarray(w_fc, dtype=np.float32)
    b_fc = np.asarray(b_fc, dtype=np.float32)
    w_proj = np.asarray(w_proj, dtype=np.float32)
    b_proj = np.asarray(b_proj, dtype=np.float32)

    B, T, C = x.shape
    x_flat = x.reshape(-1, C)
    N = x_flat.shape[0]

    i1, i2, g1, g2 = _route(x_flat, router_w)

    idxs, gates = [], []
    for e in range(N_EXPERTS):
        mask = (i1 == e) | (i2 == e)
        idx = np.flatnonzero(mask)
        g = np.where(i1[idx] == e, g1[idx], g2[idx]).astype(np.float32)
        idxs.append(idx)
        gates.append(g)

    max_cnt = max(len(ix) for ix in idxs)
    M = max(P, ((max_cnt + P - 1) // P) * P)

    repeat = int(os.environ.get("MOE_KERNEL_REPEAT", "1"))
    nc = _build(M, repeat)

    in_maps = []
    for e in range(N_EXPERTS):
        idx = idxs[e]
        xg = np.zeros((M, C), dtype=np.float32)
        xg[: len(idx)] = x_flat[idx]
        xT = np.ascontiguousarray(xg.T).reshape(KT, P, M).astype(np.float16)
        in_maps.append({
            "xT": xT,
            "w_fc": w_fc[e].reshape(KT, P, D_FF).astype(np.float16),
            "b_fcT": np.ascontiguousarray(b_fc[e].reshape(FT, P).T),
            "w_proj": w_proj[e].reshape(FT, P, N_EMBD).astype(np.float16),
        })

    res = run_bass_kernel_spmd(nc, in_maps, core_ids=list(range(N_CORES)))

    out_flat = np.zeros((N, C), dtype=np.float32)
    for e in range(N_EXPERTS):
        idx = idxs[e]
        yT = res.results[e]["outT"].reshape(C, M)       # [C, M]
        y = yT.T[: len(idx)]                            # [n_e, C]
        out_flat[idx] += gates[e][:, None] * (y + b_proj[e])

    return out_flat.reshape(B, T, C)
